# revision 37
# baseline (speedup 1.0000x reference)
"""Trainium2 Bass kernel for a pre-norm transformer encoder layer (SwiGLU FFN).

Shapes (hardcoded): x [2, 2048, 768], mask [2, 2048, 2048] int32,
wq/wk/wv/wo [768, 768], w1/w3 [3072, 768], w2 [768, 3072], g_attn/g_ffn [768].

Sharding: 8 cores = 2 batch x 4 query-slices of 512 tokens. Each core
computes K/V for its full batch element (replicated within the group of 4)
and attention + FFN for its own 512 tokens. No collectives.

On-device layout is feature-major ("transposed"): activations [D, tokens].
All matmuls run in bf16 with fp32 PSUM accumulation.

Optimizations over the first working version: bf16 x input; K/Q projected
from unnormalized x with rstd folded in at PSUM evacuation (keeps the PE
stream free of the rmsnorm chain); per-pair softmax normalization (DVE
fast-reciprocal via an SBUF bounce + gpsimd broadcast), emission deferred
into the next pair's g-loop so it never heads the DVE FIFO; head-paired wo
projection (full 128-contract); even/odd score matmuls issued adjacently for
row-tile concurrency; FFN weights prefetched during attention; small
keep-warm matmul blocks bridge the dependency-chain bubbles at phase
transitions so the PE HAM clock gate stays at 2.4 GHz; the K/Q/V
projections and all three FFN matmuls run in fp8e4m3 DoubleRow mode
(2 k-subtiles per matmul; weights scaled x64 into fp8 range and unscaled at
PSUM evacuation / via host-side folds into wo and the final 1/1024 residual
scale); the gpsimd broadcast path is warmed up in stage 1.
"""
import os
import sys

for _p in ("/opt/trn_rl_repo", "/root/.axon_site/_ro/trn_rl_repo"):
    if os.path.isdir(_p) and _p not in sys.path:
        sys.path.append(_p)

import numpy as np
import ml_dtypes

import concourse.bacc as bacc
import concourse.tile as tile
from concourse import mybir

F32 = mybir.dt.float32
BF16 = mybir.dt.bfloat16
F8 = mybir.dt.float8e4
AF = mybir.ActivationFunctionType

B, S, D, H = 2, 2048, 768, 12
DK = D // H            # 64
F = 4 * D              # 3072
T = 512                # local query tokens per core
NCH = D // 128         # 6 feature chunks
NFC = F // 128         # 24 FFN chunks
NKT = S // 128         # 16 key tiles
NQT = S // T           # 4 query slices per batch element
NPC = H // 2           # 6 head pairs
EPS = 1e-5


def build_nc():
    nc = bacc.Bacc("TRN2", target_bir_lowering=False, debug=False, num_devices=8)

    xT = nc.dram_tensor("xT", [NCH, 128, S], BF16, kind="ExternalInput").ap()
    maskT = nc.dram_tensor("maskT", [128, NKT * T], BF16, kind="ExternalInput").ap()
    xT8 = nc.dram_tensor("xT8", [NCH, 128, S], F8, kind="ExternalInput").ap()
    wqT = nc.dram_tensor("wqT", [NCH, 128, D], F8, kind="ExternalInput").ap()
    wkT = nc.dram_tensor("wkT", [NCH, 128, D], F8, kind="ExternalInput").ap()
    wvT = nc.dram_tensor("wvT", [128, NCH * D], F8, kind="ExternalInput").ap()
    woT = nc.dram_tensor("woT", [NPC, 128, D], BF16, kind="ExternalInput").ap()
    w1T = nc.dram_tensor("w1T", [NFC, 128, D], F8, kind="ExternalInput").ap()
    w3T = nc.dram_tensor("w3T", [NFC, 128, D], F8, kind="ExternalInput").ap()
    w2T = nc.dram_tensor("w2T", [NCH, 128, F], F8, kind="ExternalInput").ap()
    ones16 = nc.dram_tensor("ones16", [128, 128], BF16, kind="ExternalInput").ap()

    outT = nc.dram_tensor("outT", [NCH, 128, T], F32, kind="ExternalOutput").ap()
    warm_out = nc.dram_tensor("warm_out", [3, 128, T], BF16,
                              kind="ExternalOutput").ap()

    with tile.TileContext(nc) as tc:
        with tc.tile_pool(name="glob", bufs=1) as Pg:
            ones16_t = Pg.tile([128, 128], BF16, name="ones16_t")
            eps_t = Pg.tile([128, 1], F32, name="eps_t")
            xloc = [Pg.tile([128, T], BF16, name=f"xloc{c}") for c in range(NCH)]
            hT = [Pg.tile([128, T], F32, name=f"hT{c}") for c in range(NCH)]
            attnP = [Pg.tile([128, T], BF16, name=f"attnP{p}") for p in range(NPC)]
            hn8 = Pg.tile([128, NCH * T], F8, name="hn8")
            # first NPRE FFN chunks prefetched during attention; rest streamed
            NPRE = 12
            w1t = [Pg.tile([128, D], F8, name=f"w1_{f}") for f in range(NPRE)]
            w3t = [Pg.tile([128, D], F8, name=f"w3_{f}") for f in range(NPRE)]

            with tc.tile_pool(name="attn", bufs=1) as Pa:
                KT = [Pa.tile([128, S], BF16, name=f"KT{c}") for c in range(NCH)]
                QT = [Pa.tile([128, T], BF16, name=f"QT{c}") for c in range(NCH)]
                VA = [Pa.tile([128, H * (DK + 1)], BF16, name=f"VA{t}")
                      for t in range(NKT)]
                maskT_t = Pa.tile([128, NKT * T], BF16, name="maskT_t")

                # ---------------- stage 1: rmsnorm + Q/K/V projections --------
                with (
                    tc.tile_pool(name="s1", bufs=1) as P1,
                    tc.tile_pool(name="ps1", bufs=1, space="PSUM") as PS1,
                ):
                    wq_t = [P1.tile([128, D], F8, name=f"wq{c}") for c in range(NCH)]
                    wk_t = [P1.tile([128, D], F8, name=f"wk{c}") for c in range(NCH)]
                    wv8 = P1.tile([128, NCH * D], F8, name="wv8")

                    # the ones columns of VA are set once, values filled per tile
                    for t in range(NKT):
                        nc.vector.memset(
                            VA[t][:].rearrange("p (h e) -> p h e",
                                               e=DK + 1)[:, :, DK:DK + 1], 1.0)

                    for qt in range(NQT):
                        sl = slice(qt * T, (qt + 1) * T)
                        xq = [P1.tile([128, T], BF16, name=f"xq{c}", tag=f"xq{c}",
                                      bufs=2) for c in range(NCH)]
                        if qt == 0:
                            nc.sync.dma_start(ones16_t[:], ones16)
                            nc.vector.memset(eps_t[:], EPS)
                        for c in range(NCH):
                            nc.sync.dma_start(xq[c][:], xT[c][:, sl])
                        xq8 = P1.tile([128, NCH * T], F8, tag="xq8", bufs=2,
                                      name=f"xq8_{qt}")
                        for c in range(NCH):
                            nc.sync.dma_start(xq8[:, c * T:(c + 1) * T],
                                              xT8[c][:, sl])
                        if qt == 0:
                            for c in range(NCH):
                                nc.sync.dma_start(wk_t[c][:], wkT[c])
                            for c in range(NCH):
                                nc.sync.dma_start(wq_t[c][:], wqT[c])
                            nc.sync.dma_start(wv8[:], wvT)
                            nc.sync.dma_start(maskT_t[:], maskT)
                        ps_ms = PS1.tile([128, T], F32, tag="ps_ms", name="ps_ms")
                        for c in range(NCH):
                            sq = P1.tile([128, T], BF16, tag="sq", bufs=2,
                                         name=f"sq{qt}_{c}")
                            nc.vector.tensor_mul(sq[:], xq[c][:], xq[c][:])
                            nc.tensor.matmul(ps_ms[:], ones16_t[:], sq[:],
                                             start=(c == 0), stop=(c == NCH - 1))
                        sstd = P1.tile([128, T], F32, tag="sstd", bufs=2,
                                       name=f"sstd{qt}")
                        nc.scalar.activation(sstd[:], ps_ms[:], AF.Sqrt,
                                             bias=eps_t[:], scale=1.0 / D)
                        rstd = P1.tile([128, T], F32, tag="rstd", bufs=2,
                                       name=f"rstd{qt}")
                        nc.vector.reciprocal_approx_fast(rstd[:], sstd[:])

                        local = (qt == 0)
                        if local:
                            for c in range(NCH):
                                nc.scalar.copy(xloc[c][:], xq[c][:])
                            # warm up the gpsimd broadcast path: its first
                            # dispatch costs ~5us and would stall attention
                            bcw = P1.tile([DK, T], F32, name="bcw")
                            nc.gpsimd.partition_broadcast(bcw[:],
                                                          sstd[0:1, :])
                            wbc = P1.tile([1, T], BF16, name="wbc")
                            nc.vector.tensor_copy(wbc[:], bcw[0:1, :])
                            nc.sync.dma_start(warm_out[2][0:1, :], wbc[:])
                        # K/Q projections on UNnormalized x; rstd folded in at
                        # PSUM evacuation (keeps the PE stream free of the
                        # rmsnorm dependency chain).
                        xq8r = xq8[:].rearrange("p (c t) -> p c t", t=T)
                        for do in range(NCH):
                            wkr = wk_t[do][:].rearrange("p (c m) -> p c m", m=128)
                            ps_k = PS1.tile([128, T], F32, tag="ps_k", bufs=3,
                                            name=f"ps_k{qt}_{do}")
                            for j in range(NCH // 2):
                                js = slice(2 * j, 2 * j + 2)
                                nc.tensor.matmul(
                                    ps_k[:], wkr[:, js, :], xq8r[:, js, :],
                                    start=(j == 0), stop=(j == 2),
                                    perf_mode=mybir.MatmulPerfMode.DoubleRow)
                            nc.vector.scalar_tensor_tensor(
                                KT[do][:, sl], ps_k[:], 1.0 / 64, rstd[:],
                                mybir.AluOpType.mult, mybir.AluOpType.mult)
                            if local:
                                wqr = wq_t[do][:].rearrange("p (c m) -> p c m",
                                                            m=128)
                                ps_q = PS1.tile([128, T], F32, tag="ps_v",
                                                bufs=2, name=f"ps_q{do}")
                                for j in range(NCH // 2):
                                    js = slice(2 * j, 2 * j + 2)
                                    nc.tensor.matmul(
                                        ps_q[:], wqr[:, js, :], xq8r[:, js, :],
                                        start=(j == 0), stop=(j == 2),
                                        perf_mode=mybir.MatmulPerfMode.DoubleRow)
                                nc.vector.scalar_tensor_tensor(
                                    QT[do][:], ps_q[:], 1.0 / 64, rstd[:],
                                    mybir.AluOpType.mult, mybir.AluOpType.mult)
                        # normalized tokens (fp8, packed c-major) for V
                        xn8 = P1.tile([128, NCH * T], F8, tag="xn8", bufs=2,
                                      name=f"xn8_{qt}")
                        for c in range(NCH):
                            nc.vector.tensor_mul(xn8[:, c * T:(c + 1) * T],
                                                 xq[c][:], rstd[:])
                        xn8r = xn8[:].rearrange("p (c t) -> p c t", t=T)
                        wv8r = wv8[:].rearrange("p (c n) -> p c n", n=D)
                        # V projection: token-major tiles with ones columns.
                        # V carries the x64 fp8 weight scale; wo is pre-divided
                        for tt in range(4):
                            gt = qt * 4 + tt
                            ps_v = PS1.tile([128, D], F32, tag="ps_v", bufs=2,
                                            name=f"ps_v{gt}")
                            tsl = slice(tt * 128, (tt + 1) * 128)
                            for j in range(NCH // 2):
                                js = slice(2 * j, 2 * j + 2)
                                nc.tensor.matmul(
                                    ps_v[:, 0:512], xn8r[:, js, tsl],
                                    wv8r[:, js, 0:512],
                                    start=(j == 0), stop=(j == 2),
                                    perf_mode=mybir.MatmulPerfMode.DoubleRow)
                                nc.tensor.matmul(
                                    ps_v[:, 512:768], xn8r[:, js, tsl],
                                    wv8r[:, js, 512:768],
                                    start=(j == 0), stop=(j == 2),
                                    perf_mode=mybir.MatmulPerfMode.DoubleRow)
                            nc.vector.tensor_copy(
                                VA[gt][:].rearrange("p (h e) -> p h e",
                                                    e=DK + 1)[:, :, 0:DK],
                                ps_v[:].rearrange("p (h d) -> p h d", d=DK))

                # ---------------- stage 2: attention ------------------------
                # prefetch FFN weights while attention runs
                for f in range(NPRE):
                    nc.sync.dma_start(w1t[f][:], w1T[f])
                    nc.sync.dma_start(w3t[f][:], w3T[f])

                with (
                    tc.tile_pool(name="s2", bufs=1) as P2,
                    tc.tile_pool(name="ps2", bufs=1, space="PSUM") as PS2,
                ):
                    wo_t = [P2.tile([128, D], BF16, name=f"wo{p}")
                            for p in range(NPC)]
                    for p in range(NPC):
                        nc.sync.dma_start(wo_t[p][:], woT[p])

                    def norm_steps(pc, acc):
                        # softmax scale as 4 micro-steps (2 per head) so the
                        # DVE queue never carries the whole chain as one blob
                        # ahead of the next pair's masked-probs multiplies
                        steps = []
                        for h in (2 * pc, 2 * pc + 1):
                            def s1(h=h):
                                srow = P2.tile([1, T], F32, tag="srow", bufs=2,
                                               name=f"srow{h}")
                                nc.vector.tensor_copy(srow[:],
                                                      acc[h][DK:DK + 1, :])
                                rcp = P2.tile([1, T], F32, tag="rcp", bufs=2,
                                              name=f"rcp{h}")
                                nc.vector.reciprocal_approx_fast(rcp[:], srow[:])
                                bc = P2.tile([DK, T], F32, tag="bc", bufs=2,
                                             name=f"bc{h}")
                                nc.gpsimd.partition_broadcast(bc[:], rcp[:])
                                return bc
                            def s2(h=h, pc=pc):
                                r0 = (h % 2) * DK
                                nc.vector.tensor_mul(attnP[pc][r0:r0 + DK, :],
                                                     acc[h][0:DK, :],
                                                     bcs[h][:])
                            steps.append(((h, s1), (None, s2)))
                        # both s1 steps first: each broadcast gets two g-steps
                        # of slack before its multiply enters the DVE queue
                        return [steps[0][0], steps[1][0],
                                steps[0][1], steps[1][1]]

                    bcs = {}
                    pending = []
                    for pc in range(NPC):
                        he, ho = 2 * pc, 2 * pc + 1
                        acc = {h: PS2.tile([128, T], F32, tag="acc", bufs=4,
                                           name=f"acc{h}") for h in (he, ho)}
                        for g in range(8):
                            if 2 <= g <= 5 and pending:
                                h, step = pending.pop(0)
                                r = step()
                                if h is not None:
                                    bcs[h] = r
                            ps = {h: PS2.tile([128, 1024], F32, tag="psc",
                                              bufs=2, name=f"psc{h}_{g}")
                                  for h in (he, ho)}
                            # keep-warm pre-matmuls: fill the ACT-paced PE idle
                            # slots; overwritten by the real score matmuls via
                            # the start=True bank clear, so results are unused
                            npre = 1 if pc < 3 else 0
                            for h in (he, ho):
                                for w in range(npre):
                                    nc.tensor.matmul(
                                        ps[h][:, w * T:(w + 1) * T], ones16_t[:],
                                        xloc[(g + w) % NCH][:],
                                        start=True, stop=True)
                            # adjacent even/odd issue -> row-tile concurrency
                            for j in range(2):
                                kt = 2 * g + j
                                ksl = slice(kt * 128, (kt + 1) * 128)
                                for h in (he, ho):
                                    r0 = (h % 2) * DK
                                    nc.tensor.matmul(
                                        ps[h][:, j * T:(j + 1) * T],
                                        KT[pc][r0:r0 + DK, ksl],
                                        QT[pc][r0:r0 + DK, :],
                                        start=True, stop=True)
                            for h in (he, ho):
                                pr = P2.tile([128, 1024], BF16, tag="probs",
                                             bufs=4, name=f"probs{h}_{g}")
                                nc.scalar.activation(pr[:], ps[h][:], AF.Exp)
                                prm = P2.tile([128, 1024], BF16, tag="probsm",
                                              bufs=4, name=f"probsm{h}_{g}")
                                nc.vector.tensor_mul(
                                    prm[:], pr[:],
                                    maskT_t[:, g * 1024:(g + 1) * 1024])
                                for j in range(2):
                                    kt = 2 * g + j
                                    nc.tensor.matmul(
                                        acc[h][0:DK + 1, :],
                                        VA[kt][:, h * (DK + 1):(h + 1) * (DK + 1)],
                                        prm[:, j * T:(j + 1) * T],
                                        start=(g == 0 and j == 0),
                                        stop=(g == 7 and j == 1))
                        pending = norm_steps(pc, acc)
                    # keep-warm filler: bridges the final normalize chain so
                    # the PE clock gate never sees an idle MID window
                    wps = PS2.tile([128, 1024], F32, tag="psc", bufs=2,
                                   name="warm_ps2")
                    for i in range(16):
                        nc.tensor.matmul(wps[:, 0:T], ones16_t[:], xloc[i % NCH][:],
                                         start=(i == 0), stop=(i == 15))
                    wsb = P2.tile([128, T], BF16, name="warm_sb2")
                    nc.vector.tensor_copy(wsb[:], wps[:, 0:T])
                    nc.sync.dma_start(warm_out[0], wsb[:])
                    for h, step in pending:
                        r = step()
                        if h is not None:
                            bcs[h] = r

                    # wo projection (head pairs, full contract) + residual
                    ps_ms2 = PS2.tile([128, T], F32, tag="acc", bufs=4,
                                      name="ps_ms2")
                    for do in range(NCH):
                        ps_h2 = PS2.tile([128, 1024], F32, tag="psc", bufs=2,
                                         name=f"ps_h2_{do}")
                        for p in range(NPC):
                            nc.tensor.matmul(
                                ps_h2[:, 0:T],
                                wo_t[p][:, do * 128:(do + 1) * 128],
                                attnP[p][:], start=(p == 0), stop=(p == NPC - 1))
                        nc.vector.tensor_add(hT[do][:], ps_h2[:, 0:T], xloc[do][:])
                        sqh = P2.tile([128, T], BF16, tag="sqh", bufs=2,
                                      name=f"sqh{do}")
                        nc.vector.tensor_mul(sqh[:], hT[do][:], hT[do][:])
                        nc.tensor.matmul(ps_ms2[:], ones16_t[:], sqh[:],
                                         start=(do == 0), stop=(do == NCH - 1))
                    sstd2 = P2.tile([128, T], F32, name="sstd2")
                    nc.scalar.activation(sstd2[:], ps_ms2[:], AF.Sqrt,
                                         bias=eps_t[:], scale=1.0 / D)
                    rstd2 = P2.tile([128, T], F32, name="rstd2")
                    nc.vector.reciprocal_approx_fast(rstd2[:], sstd2[:])
                    for c in range(NCH):
                        nc.vector.tensor_mul(hn8[:, c * T:(c + 1) * T],
                                             hT[c][:], rstd2[:])

            # ------------- stage 3: SwiGLU FFN ------------------------------
            with (
                tc.tile_pool(name="s4", bufs=1) as P4,
                tc.tile_pool(name="ps4", bufs=1, space="PSUM") as PS4,
            ):
                wps4 = PS4.tile([128, T], F32, tag="warm", bufs=1,
                                name="warm_ps4")
                for i in range(12):
                    nc.tensor.matmul(wps4[:], ones16_t[:], xloc[i % NCH][:],
                                     start=(i == 0), stop=(i == 11))
                wsb4 = P4.tile([128, T], BF16, name="warm_sb4")
                nc.vector.tensor_copy(wsb4[:], wps4[:])
                nc.sync.dma_start(warm_out[1], wsb4[:])
                w2t = [P4.tile([128, F], F8, tag="w2_t", bufs=2,
                               name=f"w2_{do}") for do in range(NCH)]
                for do in range(NCH):
                    nc.sync.dma_start(w2t[do][:], w2T[do])
                prod8 = P4.tile([128, NFC * T], F8, name="prod8")
                for f in range(NFC):
                    if f < NPRE:
                        w1f, w3f = w1t[f], w3t[f]
                    else:
                        w1f = P4.tile([128, D], F8, tag="w1s", bufs=6,
                                      name=f"w1s{f}")
                        w3f = P4.tile([128, D], F8, tag="w3s", bufs=6,
                                      name=f"w3s{f}")
                        nc.sync.dma_start(w1f[:], w1T[f])
                        nc.sync.dma_start(w3f[:], w3T[f])
                    ps_u = PS4.tile([128, T], F32, tag="ps_u", bufs=2,
                                    name=f"ps_u{f}")
                    ps_w = PS4.tile([128, T], F32, tag="ps_w", bufs=2,
                                    name=f"ps_w{f}")
                    w1r = w1f[:].rearrange("p (c m) -> p c m", m=128)
                    w3r = w3f[:].rearrange("p (c m) -> p c m", m=128)
                    hnr = hn8[:].rearrange("p (c t) -> p c t", t=T)
                    for j in range(NCH // 2):
                        js = slice(2 * j, 2 * j + 2)
                        nc.tensor.matmul(ps_u[:], w1r[:, js, :], hnr[:, js, :],
                                         start=(j == 0), stop=(j == 2),
                                         perf_mode=mybir.MatmulPerfMode.DoubleRow)
                        nc.tensor.matmul(ps_w[:], w3r[:, js, :], hnr[:, js, :],
                                         start=(j == 0), stop=(j == 2),
                                         perf_mode=mybir.MatmulPerfMode.DoubleRow)
                    # weights carry a x64 fp8 range scale; silu's input scale
                    # undoes it for u, w2 is pre-divided to undo it for w
                    silu = P4.tile([128, T], BF16, tag="silu", bufs=2,
                                   name=f"silu{f}")
                    if os.environ.get("BASS_SIM_SILU") == "1":
                        # CoreSim has no Silu; emulate as u*sigmoid(u)
                        nc.scalar.activation(silu[:], ps_u[:], AF.Sigmoid,
                                             scale=1.0 / 64)
                        nc.vector.tensor_mul(silu[:], silu[:], ps_u[:])
                        nc.vector.scalar_tensor_tensor(
                            prod8[:, f * T:(f + 1) * T], silu[:], 0.25 / 64,
                            ps_w[:], mybir.AluOpType.mult,
                            mybir.AluOpType.mult)
                    else:
                        nc.scalar.activation(silu[:], ps_u[:], AF.Silu,
                                             scale=1.0 / 64)
                        # prod kept at x16 scale so fp8 outliers cannot overflow
                        nc.vector.scalar_tensor_tensor(
                            prod8[:, f * T:(f + 1) * T], silu[:], 0.25,
                            ps_w[:], mybir.AluOpType.mult,
                            mybir.AluOpType.mult)

                prod8r = prod8[:].rearrange("p (f t) -> p f t", t=T)
                for do in range(NCH):
                    w2r = w2t[do][:].rearrange("p (f m) -> p f m", m=128)
                    ps_y = PS4.tile([128, T], F32, tag="ps_y", bufs=2,
                                    name=f"ps_y{do}")
                    for j in range(NFC // 2):
                        js = slice(2 * j, 2 * j + 2)
                        nc.tensor.matmul(ps_y[:], w2r[:, js, :],
                                         prod8r[:, js, :],
                                         start=(j == 0), stop=(j == NFC // 2 - 1),
                                         perf_mode=mybir.MatmulPerfMode.DoubleRow)
                    outt = P4.tile([128, T], F32, tag="outt", bufs=2,
                                   name=f"outt{do}")
                    # w2 carries x64, prod x16 -> undo 1/1024
                    nc.vector.scalar_tensor_tensor(
                        outt[:], ps_y[:], 1.0 / 1024, hT[do][:],
                        mybir.AluOpType.mult, mybir.AluOpType.add)
                    nc.sync.dma_start(outT[do], outt[:])

    nc.compile()
    return nc


def prep_inputs(x, mask, wq, wk, wv, wo, w1, w2, w3, g_attn, g_ffn):
    """Build the 8 per-core input maps (host-side sharding + layout)."""
    bf = ml_dtypes.bfloat16
    f8 = ml_dtypes.float8_e4m3
    # K/Q weights: fp8 DoubleRow layout [do, p, (c m)], scaled x64 into fp8
    # range (unscaled at PSUM evacuation); wq also folds 1/sqrt(dk)
    wq_s = 64.0 * wq * (1.0 / np.sqrt(DK))
    wqTe = np.ascontiguousarray(
        (wq_s * g_attn[None, :]).T.reshape(NCH, 128, NCH, 128)
        .transpose(2, 1, 0, 3).reshape(NCH, 128, D)).astype(f8)
    wkTe = np.ascontiguousarray(
        (64.0 * wk * g_attn[None, :]).T.reshape(NCH, 128, NCH, 128)
        .transpose(2, 1, 0, 3).reshape(NCH, 128, D)).astype(f8)
    # V weights: fp8 [p, (c n)]; the x64 rides through V and is undone by wo/64
    wvTe = np.ascontiguousarray(
        (64.0 * wv * g_attn[None, :]).T.reshape(NCH, 128, D)
        .transpose(1, 0, 2).reshape(128, NCH * D)).astype(f8)
    woTe = np.ascontiguousarray((wo / 64.0).T.reshape(NPC, 128, D)).astype(bf)
    f8 = ml_dtypes.float8_e4m3
    w1Te = np.ascontiguousarray(
        (64.0 * w1 * g_ffn[None, :]).T.reshape(NCH, 128, NFC, 128)
        .transpose(2, 1, 0, 3).reshape(NFC, 128, D)).astype(f8)
    w3Te = np.ascontiguousarray(
        (64.0 * w3 * g_ffn[None, :]).T.reshape(NCH, 128, NFC, 128)
        .transpose(2, 1, 0, 3).reshape(NFC, 128, D)).astype(f8)
    w2Te = np.ascontiguousarray(
        (64.0 * w2).T.reshape(NFC, 128, NCH, 128).transpose(2, 1, 0, 3)
        .reshape(NCH, 128, F)).astype(f8)
    ones16 = np.ones((128, 128), bf)

    in_maps = []
    for core in range(8):
        b, qt = core // NQT, core % NQT
        # rotate tokens so the local 512-query slice is always quarter 0
        order = (np.arange(S) + qt * T) % S
        xb = x[b][order]                       # [S, D] rotated
        xTe = np.ascontiguousarray(xb.T.reshape(NCH, 128, S)).astype(bf)
        xTe8 = np.ascontiguousarray(xb.T.reshape(NCH, 128, S)).astype(f8)
        # maskT[p, kt*T + q] = mask[b, qt*T + q, k] with k = kt*128 + p in
        # ROTATED key order (keys follow the same rotation as tokens).
        msl = mask[b, qt * T:(qt + 1) * T][:, order]     # [T(q), S(k)] rotated
        maskTe = np.ascontiguousarray(
            msl.T.reshape(NKT, 128, T).transpose(1, 0, 2)
            .reshape(128, NKT * T)).astype(bf)
        in_maps.append({
            "xT": xTe, "xT8": xTe8, "maskT": maskTe,
            "wqT": wqTe, "wkT": wkTe, "wvT": wvTe, "woT": woTe,
            "w1T": w1Te, "w3T": w3Te, "w2T": w2Te,
            "ones16": ones16,
        })
    return in_maps


_NC_CACHE = None


def get_nc():
    global _NC_CACHE
    if _NC_CACHE is None:
        _NC_CACHE = build_nc()
    return _NC_CACHE


def gather_output(results):
    out = np.empty((B, S, D), np.float32)
    for core in range(8):
        b, qt = core // NQT, core % NQT
        o = results[core]["outT"]              # [NCH, 128, T]
        out[b, qt * T:(qt + 1) * T, :] = o.reshape(D, T).T
    return out


def kernel(**inputs):
    from concourse.bass_utils import run_bass_kernel_spmd
    in_maps = prep_inputs(
        np.asarray(inputs["x"]), np.asarray(inputs["mask"]),
        np.asarray(inputs["wq"]), np.asarray(inputs["wk"]),
        np.asarray(inputs["wv"]), np.asarray(inputs["wo"]),
        np.asarray(inputs["w1"]), np.asarray(inputs["w2"]),
        np.asarray(inputs["w3"]),
        np.asarray(inputs["g_attn"]), np.asarray(inputs["g_ffn"]))
    nc = get_nc()
    res = run_bass_kernel_spmd(nc, in_maps, core_ids=list(range(8)))
    return gather_output(res.results)


# revision 38
# speedup vs baseline: 1.0212x; 1.0212x over previous
"""Trainium2 Bass kernel for a pre-norm transformer encoder layer (SwiGLU FFN).

Shapes (hardcoded): x [2, 2048, 768], mask [2, 2048, 2048] int32,
wq/wk/wv/wo [768, 768], w1/w3 [3072, 768], w2 [768, 3072], g_attn/g_ffn [768].

Sharding: 8 cores = 2 batch x 4 query-slices of 512 tokens. Each core
computes K/V for its full batch element (replicated within the group of 4)
and attention + FFN for its own 512 tokens. No collectives.

On-device layout is feature-major ("transposed"): activations [D, tokens].
All matmuls run in bf16 with fp32 PSUM accumulation.

Optimizations over the first working version: bf16 x input; K/Q projected
from unnormalized x with rstd folded in at PSUM evacuation (keeps the PE
stream free of the rmsnorm chain); per-pair softmax normalization (DVE
fast-reciprocal via an SBUF bounce + gpsimd broadcast), emission deferred
into the next pair's g-loop so it never heads the DVE FIFO; head-paired wo
projection (full 128-contract); even/odd score matmuls issued adjacently for
row-tile concurrency; FFN weights prefetched during attention; small
keep-warm matmul blocks bridge the dependency-chain bubbles at phase
transitions so the PE HAM clock gate stays at 2.4 GHz; the K/Q/V
projections and all three FFN matmuls run in fp8e4m3 DoubleRow mode
(2 k-subtiles per matmul; weights scaled x64 into fp8 range and unscaled at
PSUM evacuation / via host-side folds into wo and the final 1/1024 residual
scale); the gpsimd broadcast path is warmed up in stage 1.
"""
import os
import sys

for _p in ("/opt/trn_rl_repo", "/root/.axon_site/_ro/trn_rl_repo"):
    if os.path.isdir(_p) and _p not in sys.path:
        sys.path.append(_p)

import numpy as np
import ml_dtypes

import concourse.bacc as bacc
import concourse.tile as tile
from concourse import mybir

F32 = mybir.dt.float32
BF16 = mybir.dt.bfloat16
F8 = mybir.dt.float8e4
AF = mybir.ActivationFunctionType

B, S, D, H = 2, 2048, 768, 12
DK = D // H            # 64
F = 4 * D              # 3072
T = 512                # local query tokens per core
NCH = D // 128         # 6 feature chunks
NFC = F // 128         # 24 FFN chunks
NKT = S // 128         # 16 key tiles
NQT = S // T           # 4 query slices per batch element
NPC = H // 2           # 6 head pairs
EPS = 1e-5


def build_nc():
    nc = bacc.Bacc("TRN2", target_bir_lowering=False, debug=False, num_devices=8)

    xT = nc.dram_tensor("xT", [NCH, 128, S], BF16, kind="ExternalInput").ap()
    maskT = nc.dram_tensor("maskT", [128, NKT * T], BF16, kind="ExternalInput").ap()
    xT8 = nc.dram_tensor("xT8", [NCH, 128, S], F8, kind="ExternalInput").ap()
    wqT = nc.dram_tensor("wqT", [NCH, 128, D], F8, kind="ExternalInput").ap()
    wkT = nc.dram_tensor("wkT", [NCH, 128, D], F8, kind="ExternalInput").ap()
    wvT = nc.dram_tensor("wvT", [128, NCH * D], F8, kind="ExternalInput").ap()
    woT = nc.dram_tensor("woT", [NPC, 128, D], BF16, kind="ExternalInput").ap()
    w1T = nc.dram_tensor("w1T", [NFC, 128, D], F8, kind="ExternalInput").ap()
    w3T = nc.dram_tensor("w3T", [NFC, 128, D], F8, kind="ExternalInput").ap()
    w2T = nc.dram_tensor("w2T", [NCH, 128, F], F8, kind="ExternalInput").ap()
    ones16 = nc.dram_tensor("ones16", [128, 128], BF16, kind="ExternalInput").ap()

    outT = nc.dram_tensor("outT", [NCH, 128, T], F32, kind="ExternalOutput").ap()
    warm_out = nc.dram_tensor("warm_out", [3, 128, T], BF16,
                              kind="ExternalOutput").ap()

    with tile.TileContext(nc) as tc:
        with tc.tile_pool(name="glob", bufs=1) as Pg:
            ones16_t = Pg.tile([128, 128], BF16, name="ones16_t")
            eps_t = Pg.tile([128, 1], F32, name="eps_t")
            xloc = [Pg.tile([128, T], BF16, name=f"xloc{c}") for c in range(NCH)]
            hT = [Pg.tile([128, T], F32, name=f"hT{c}") for c in range(NCH)]
            attnP = [Pg.tile([128, T], BF16, name=f"attnP{p}") for p in range(NPC)]
            hn8 = Pg.tile([128, NCH * T], F8, name="hn8")
            # first NPRE FFN chunks prefetched during attention; rest streamed
            NPRE = 12
            w1t = [Pg.tile([128, D], F8, name=f"w1_{f}") for f in range(NPRE)]
            w3t = [Pg.tile([128, D], F8, name=f"w3_{f}") for f in range(NPRE)]

            with tc.tile_pool(name="attn", bufs=1) as Pa:
                KT = [Pa.tile([128, S], BF16, name=f"KT{c}") for c in range(NCH)]
                QT = [Pa.tile([128, T], BF16, name=f"QT{c}") for c in range(NCH)]
                VA = [Pa.tile([128, H * (DK + 1)], BF16, name=f"VA{t}")
                      for t in range(NKT)]
                maskT_t = Pa.tile([128, NKT * T], BF16, name="maskT_t")

                # ---------------- stage 1: rmsnorm + Q/K/V projections --------
                with (
                    tc.tile_pool(name="s1", bufs=1) as P1,
                    tc.tile_pool(name="ps1", bufs=1, space="PSUM") as PS1,
                ):
                    wq_t = [P1.tile([128, D], F8, name=f"wq{c}") for c in range(NCH)]
                    wk_t = [P1.tile([128, D], F8, name=f"wk{c}") for c in range(NCH)]
                    wv8 = P1.tile([128, NCH * D], F8, name="wv8")

                    # the ones columns of VA are set once, values filled per tile
                    for t in range(NKT):
                        nc.vector.memset(
                            VA[t][:].rearrange("p (h e) -> p h e",
                                               e=DK + 1)[:, :, DK:DK + 1], 1.0)

                    for qt in range(NQT):
                        sl = slice(qt * T, (qt + 1) * T)
                        xq = [P1.tile([128, T], BF16, name=f"xq{c}", tag=f"xq{c}",
                                      bufs=2) for c in range(NCH)]
                        if qt == 0:
                            nc.sync.dma_start(ones16_t[:], ones16)
                            nc.vector.memset(eps_t[:], EPS)
                        for c in range(NCH):
                            nc.sync.dma_start(xq[c][:], xT[c][:, sl])
                        xq8 = P1.tile([128, NCH * T], F8, tag="xq8", bufs=2,
                                      name=f"xq8_{qt}")
                        for c in range(NCH):
                            nc.sync.dma_start(xq8[:, c * T:(c + 1) * T],
                                              xT8[c][:, sl])
                        if qt == 0:
                            for c in range(NCH):
                                nc.sync.dma_start(wk_t[c][:], wkT[c])
                            for c in range(NCH):
                                nc.sync.dma_start(wq_t[c][:], wqT[c])
                            nc.sync.dma_start(wv8[:], wvT)
                            nc.sync.dma_start(maskT_t[:], maskT)
                        ps_ms = PS1.tile([128, T], F32, tag="ps_ms", name="ps_ms")
                        for c in range(NCH):
                            sq = P1.tile([128, T], BF16, tag="sq", bufs=2,
                                         name=f"sq{qt}_{c}")
                            nc.vector.tensor_mul(sq[:], xq[c][:], xq[c][:])
                            nc.tensor.matmul(ps_ms[:], ones16_t[:], sq[:],
                                             start=(c == 0), stop=(c == NCH - 1))
                        lntmp = P1.tile([128, T], F32, tag="lntmp", bufs=2,
                                        name=f"lntmp{qt}")
                        nc.scalar.activation(lntmp[:], ps_ms[:], AF.Ln,
                                             bias=eps_t[:], scale=1.0 / D)
                        rstd = P1.tile([128, T], F32, tag="rstd", bufs=2,
                                       name=f"rstd{qt}")
                        nc.scalar.activation(rstd[:], lntmp[:], AF.Exp,
                                             scale=-0.5)

                        local = (qt == 0)
                        if local:
                            for c in range(NCH):
                                nc.scalar.copy(xloc[c][:], xq[c][:])
                            # warm up the gpsimd broadcast path: its first
                            # dispatch costs ~5us and would stall attention
                            bcw = P1.tile([DK, T], F32, name="bcw")
                            nc.gpsimd.partition_broadcast(bcw[:],
                                                          lntmp[0:1, :])
                            wbc = P1.tile([1, T], BF16, name="wbc")
                            nc.vector.tensor_copy(wbc[:], bcw[0:1, :])
                            nc.sync.dma_start(warm_out[2][0:1, :], wbc[:])
                        # K/Q projections on UNnormalized x; rstd folded in at
                        # PSUM evacuation (keeps the PE stream free of the
                        # rmsnorm dependency chain).
                        xq8r = xq8[:].rearrange("p (c t) -> p c t", t=T)
                        for do in range(NCH):
                            wkr = wk_t[do][:].rearrange("p (c m) -> p c m", m=128)
                            ps_k = PS1.tile([128, T], F32, tag="ps_k", bufs=3,
                                            name=f"ps_k{qt}_{do}")
                            for j in range(NCH // 2):
                                js = slice(2 * j, 2 * j + 2)
                                nc.tensor.matmul(
                                    ps_k[:], wkr[:, js, :], xq8r[:, js, :],
                                    start=(j == 0), stop=(j == 2),
                                    perf_mode=mybir.MatmulPerfMode.DoubleRow)
                            nc.vector.scalar_tensor_tensor(
                                KT[do][:, sl], ps_k[:], 1.0 / 64, rstd[:],
                                mybir.AluOpType.mult, mybir.AluOpType.mult)
                            if local:
                                wqr = wq_t[do][:].rearrange("p (c m) -> p c m",
                                                            m=128)
                                ps_q = PS1.tile([128, T], F32, tag="ps_v",
                                                bufs=2, name=f"ps_q{do}")
                                for j in range(NCH // 2):
                                    js = slice(2 * j, 2 * j + 2)
                                    nc.tensor.matmul(
                                        ps_q[:], wqr[:, js, :], xq8r[:, js, :],
                                        start=(j == 0), stop=(j == 2),
                                        perf_mode=mybir.MatmulPerfMode.DoubleRow)
                                nc.vector.scalar_tensor_tensor(
                                    QT[do][:], ps_q[:], 1.0 / 64, rstd[:],
                                    mybir.AluOpType.mult, mybir.AluOpType.mult)
                        # normalized tokens (fp8, packed c-major) for V
                        xn8 = P1.tile([128, NCH * T], F8, tag="xn8", bufs=2,
                                      name=f"xn8_{qt}")
                        for c in range(NCH):
                            nc.vector.tensor_mul(xn8[:, c * T:(c + 1) * T],
                                                 xq[c][:], rstd[:])
                        xn8r = xn8[:].rearrange("p (c t) -> p c t", t=T)
                        wv8r = wv8[:].rearrange("p (c n) -> p c n", n=D)
                        # V projection: token-major tiles with ones columns.
                        # V carries the x64 fp8 weight scale; wo is pre-divided
                        for tt in range(4):
                            gt = qt * 4 + tt
                            ps_v = PS1.tile([128, D], F32, tag="ps_v", bufs=2,
                                            name=f"ps_v{gt}")
                            tsl = slice(tt * 128, (tt + 1) * 128)
                            for j in range(NCH // 2):
                                js = slice(2 * j, 2 * j + 2)
                                nc.tensor.matmul(
                                    ps_v[:, 0:512], xn8r[:, js, tsl],
                                    wv8r[:, js, 0:512],
                                    start=(j == 0), stop=(j == 2),
                                    perf_mode=mybir.MatmulPerfMode.DoubleRow)
                                nc.tensor.matmul(
                                    ps_v[:, 512:768], xn8r[:, js, tsl],
                                    wv8r[:, js, 512:768],
                                    start=(j == 0), stop=(j == 2),
                                    perf_mode=mybir.MatmulPerfMode.DoubleRow)
                            nc.vector.tensor_copy(
                                VA[gt][:].rearrange("p (h e) -> p h e",
                                                    e=DK + 1)[:, :, 0:DK],
                                ps_v[:].rearrange("p (h d) -> p h d", d=DK))

                # ---------------- stage 2: attention ------------------------
                # prefetch FFN weights while attention runs
                for f in range(NPRE):
                    nc.sync.dma_start(w1t[f][:], w1T[f])
                    nc.sync.dma_start(w3t[f][:], w3T[f])

                with (
                    tc.tile_pool(name="s2", bufs=1) as P2,
                    tc.tile_pool(name="ps2", bufs=1, space="PSUM") as PS2,
                ):
                    wo_t = [P2.tile([128, D], BF16, name=f"wo{p}")
                            for p in range(NPC)]
                    for p in range(NPC):
                        nc.sync.dma_start(wo_t[p][:], woT[p])

                    def norm_steps(pc, acc):
                        # softmax scale as 4 micro-steps (2 per head) so the
                        # DVE queue never carries the whole chain as one blob
                        # ahead of the next pair's masked-probs multiplies
                        steps = []
                        for h in (2 * pc, 2 * pc + 1):
                            def s1(h=h):
                                srow = P2.tile([1, T], F32, tag="srow", bufs=2,
                                               name=f"srow{h}")
                                nc.vector.tensor_copy(srow[:],
                                                      acc[h][DK:DK + 1, :])
                                rcp = P2.tile([1, T], F32, tag="rcp", bufs=2,
                                              name=f"rcp{h}")
                                nc.vector.reciprocal_approx_fast(rcp[:], srow[:])
                                bc = P2.tile([DK, T], F32, tag="bc", bufs=2,
                                             name=f"bc{h}")
                                nc.gpsimd.partition_broadcast(bc[:], rcp[:])
                                return bc
                            def s2(h=h, pc=pc):
                                r0 = (h % 2) * DK
                                nc.vector.tensor_mul(attnP[pc][r0:r0 + DK, :],
                                                     acc[h][0:DK, :],
                                                     bcs[h][:])
                            steps.append(((h, s1), (None, s2)))
                        # both s1 steps first: each broadcast gets two g-steps
                        # of slack before its multiply enters the DVE queue
                        return [steps[0][0], steps[1][0],
                                steps[0][1], steps[1][1]]

                    bcs = {}
                    pending = []
                    for pc in range(NPC):
                        he, ho = 2 * pc, 2 * pc + 1
                        acc = {h: PS2.tile([128, T], F32, tag="acc", bufs=4,
                                           name=f"acc{h}") for h in (he, ho)}
                        for g in range(8):
                            if 2 <= g <= 5 and pending:
                                h, step = pending.pop(0)
                                r = step()
                                if h is not None:
                                    bcs[h] = r
                            ps = {h: PS2.tile([128, 1024], F32, tag="psc",
                                              bufs=2, name=f"psc{h}_{g}")
                                  for h in (he, ho)}
                            # keep-warm pre-matmuls: fill the ACT-paced PE idle
                            # slots; overwritten by the real score matmuls via
                            # the start=True bank clear, so results are unused
                            npre = 1 if pc < 3 else 0
                            for h in (he, ho):
                                for w in range(npre):
                                    nc.tensor.matmul(
                                        ps[h][:, w * T:(w + 1) * T], ones16_t[:],
                                        xloc[(g + w) % NCH][:],
                                        start=True, stop=True)
                            # adjacent even/odd issue -> row-tile concurrency
                            for j in range(2):
                                kt = 2 * g + j
                                ksl = slice(kt * 128, (kt + 1) * 128)
                                for h in (he, ho):
                                    r0 = (h % 2) * DK
                                    nc.tensor.matmul(
                                        ps[h][:, j * T:(j + 1) * T],
                                        KT[pc][r0:r0 + DK, ksl],
                                        QT[pc][r0:r0 + DK, :],
                                        start=True, stop=True)
                            for h in (he, ho):
                                pr = P2.tile([128, 1024], BF16, tag="probs",
                                             bufs=4, name=f"probs{h}_{g}")
                                nc.scalar.activation(pr[:], ps[h][:], AF.Exp)
                                prm = P2.tile([128, 1024], BF16, tag="probsm",
                                              bufs=4, name=f"probsm{h}_{g}")
                                nc.vector.tensor_mul(
                                    prm[:], pr[:],
                                    maskT_t[:, g * 1024:(g + 1) * 1024])
                                for j in range(2):
                                    kt = 2 * g + j
                                    nc.tensor.matmul(
                                        acc[h][0:DK + 1, :],
                                        VA[kt][:, h * (DK + 1):(h + 1) * (DK + 1)],
                                        prm[:, j * T:(j + 1) * T],
                                        start=(g == 0 and j == 0),
                                        stop=(g == 7 and j == 1))
                        pending = norm_steps(pc, acc)
                    # keep-warm filler: bridges the final normalize chain so
                    # the PE clock gate never sees an idle MID window
                    wps = PS2.tile([128, 1024], F32, tag="psc", bufs=2,
                                   name="warm_ps2")
                    for i in range(16):
                        nc.tensor.matmul(wps[:, 0:T], ones16_t[:], xloc[i % NCH][:],
                                         start=(i == 0), stop=(i == 15))
                    wsb = P2.tile([128, T], BF16, name="warm_sb2")
                    nc.vector.tensor_copy(wsb[:], wps[:, 0:T])
                    nc.sync.dma_start(warm_out[0], wsb[:])
                    for h, step in pending:
                        r = step()
                        if h is not None:
                            bcs[h] = r

                    # wo projection (head pairs, full contract) + residual
                    ps_ms2 = PS2.tile([128, T], F32, tag="acc", bufs=4,
                                      name="ps_ms2")
                    for do in range(NCH):
                        ps_h2 = PS2.tile([128, 1024], F32, tag="psc", bufs=2,
                                         name=f"ps_h2_{do}")
                        for p in range(NPC):
                            nc.tensor.matmul(
                                ps_h2[:, 0:T],
                                wo_t[p][:, do * 128:(do + 1) * 128],
                                attnP[p][:], start=(p == 0), stop=(p == NPC - 1))
                        nc.vector.tensor_add(hT[do][:], ps_h2[:, 0:T], xloc[do][:])
                        sqh = P2.tile([128, T], BF16, tag="sqh", bufs=2,
                                      name=f"sqh{do}")
                        nc.vector.tensor_mul(sqh[:], hT[do][:], hT[do][:])
                        nc.tensor.matmul(ps_ms2[:], ones16_t[:], sqh[:],
                                         start=(do == 0), stop=(do == NCH - 1))
                    lntmp2 = P2.tile([128, T], F32, name="lntmp2")
                    nc.scalar.activation(lntmp2[:], ps_ms2[:], AF.Ln,
                                         bias=eps_t[:], scale=1.0 / D)
                    rstd2 = P2.tile([128, T], F32, name="rstd2")
                    nc.scalar.activation(rstd2[:], lntmp2[:], AF.Exp,
                                         scale=-0.5)
                    for c in range(NCH):
                        nc.vector.tensor_mul(hn8[:, c * T:(c + 1) * T],
                                             hT[c][:], rstd2[:])

            # ------------- stage 3: SwiGLU FFN ------------------------------
            with (
                tc.tile_pool(name="s4", bufs=1) as P4,
                tc.tile_pool(name="ps4", bufs=1, space="PSUM") as PS4,
            ):
                wps4 = PS4.tile([128, T], F32, tag="warm", bufs=1,
                                name="warm_ps4")
                for i in range(12):
                    nc.tensor.matmul(wps4[:], ones16_t[:], xloc[i % NCH][:],
                                     start=(i == 0), stop=(i == 11))
                wsb4 = P4.tile([128, T], BF16, name="warm_sb4")
                nc.vector.tensor_copy(wsb4[:], wps4[:])
                nc.sync.dma_start(warm_out[1], wsb4[:])
                w2t = [P4.tile([128, F], F8, tag="w2_t", bufs=2,
                               name=f"w2_{do}") for do in range(NCH)]
                for do in range(NCH):
                    nc.sync.dma_start(w2t[do][:], w2T[do])
                prod8 = P4.tile([128, NFC * T], F8, name="prod8")
                for f in range(NFC):
                    if f < NPRE:
                        w1f, w3f = w1t[f], w3t[f]
                    else:
                        w1f = P4.tile([128, D], F8, tag="w1s", bufs=6,
                                      name=f"w1s{f}")
                        w3f = P4.tile([128, D], F8, tag="w3s", bufs=6,
                                      name=f"w3s{f}")
                        nc.sync.dma_start(w1f[:], w1T[f])
                        nc.sync.dma_start(w3f[:], w3T[f])
                    ps_u = PS4.tile([128, T], F32, tag="ps_u", bufs=2,
                                    name=f"ps_u{f}")
                    ps_w = PS4.tile([128, T], F32, tag="ps_w", bufs=2,
                                    name=f"ps_w{f}")
                    w1r = w1f[:].rearrange("p (c m) -> p c m", m=128)
                    w3r = w3f[:].rearrange("p (c m) -> p c m", m=128)
                    hnr = hn8[:].rearrange("p (c t) -> p c t", t=T)
                    for j in range(NCH // 2):
                        js = slice(2 * j, 2 * j + 2)
                        nc.tensor.matmul(ps_u[:], w1r[:, js, :], hnr[:, js, :],
                                         start=(j == 0), stop=(j == 2),
                                         perf_mode=mybir.MatmulPerfMode.DoubleRow)
                        nc.tensor.matmul(ps_w[:], w3r[:, js, :], hnr[:, js, :],
                                         start=(j == 0), stop=(j == 2),
                                         perf_mode=mybir.MatmulPerfMode.DoubleRow)
                    # weights carry a x64 fp8 range scale; silu's input scale
                    # undoes it for u, w2 is pre-divided to undo it for w
                    silu = P4.tile([128, T], BF16, tag="silu", bufs=2,
                                   name=f"silu{f}")
                    if os.environ.get("BASS_SIM_SILU") == "1":
                        # CoreSim has no Silu; emulate as u*sigmoid(u)
                        nc.scalar.activation(silu[:], ps_u[:], AF.Sigmoid,
                                             scale=1.0 / 64)
                        nc.vector.tensor_mul(silu[:], silu[:], ps_u[:])
                        nc.vector.scalar_tensor_tensor(
                            prod8[:, f * T:(f + 1) * T], silu[:], 0.25 / 64,
                            ps_w[:], mybir.AluOpType.mult,
                            mybir.AluOpType.mult)
                    else:
                        nc.scalar.activation(silu[:], ps_u[:], AF.Silu,
                                             scale=1.0 / 64)
                        # prod kept at x16 scale so fp8 outliers cannot overflow
                        nc.vector.scalar_tensor_tensor(
                            prod8[:, f * T:(f + 1) * T], silu[:], 0.25,
                            ps_w[:], mybir.AluOpType.mult,
                            mybir.AluOpType.mult)

                prod8r = prod8[:].rearrange("p (f t) -> p f t", t=T)
                for do in range(NCH):
                    w2r = w2t[do][:].rearrange("p (f m) -> p f m", m=128)
                    ps_y = PS4.tile([128, T], F32, tag="ps_y", bufs=2,
                                    name=f"ps_y{do}")
                    for j in range(NFC // 2):
                        js = slice(2 * j, 2 * j + 2)
                        nc.tensor.matmul(ps_y[:], w2r[:, js, :],
                                         prod8r[:, js, :],
                                         start=(j == 0), stop=(j == NFC // 2 - 1),
                                         perf_mode=mybir.MatmulPerfMode.DoubleRow)
                    outt = P4.tile([128, T], F32, tag="outt", bufs=2,
                                   name=f"outt{do}")
                    # w2 carries x64, prod x16 -> undo 1/1024
                    nc.vector.scalar_tensor_tensor(
                        outt[:], ps_y[:], 1.0 / 1024, hT[do][:],
                        mybir.AluOpType.mult, mybir.AluOpType.add)
                    nc.sync.dma_start(outT[do], outt[:])

    nc.compile()
    return nc


def prep_inputs(x, mask, wq, wk, wv, wo, w1, w2, w3, g_attn, g_ffn):
    """Build the 8 per-core input maps (host-side sharding + layout)."""
    bf = ml_dtypes.bfloat16
    f8 = ml_dtypes.float8_e4m3
    # K/Q weights: fp8 DoubleRow layout [do, p, (c m)], scaled x64 into fp8
    # range (unscaled at PSUM evacuation); wq also folds 1/sqrt(dk)
    wq_s = 64.0 * wq * (1.0 / np.sqrt(DK))
    wqTe = np.ascontiguousarray(
        (wq_s * g_attn[None, :]).T.reshape(NCH, 128, NCH, 128)
        .transpose(2, 1, 0, 3).reshape(NCH, 128, D)).astype(f8)
    wkTe = np.ascontiguousarray(
        (64.0 * wk * g_attn[None, :]).T.reshape(NCH, 128, NCH, 128)
        .transpose(2, 1, 0, 3).reshape(NCH, 128, D)).astype(f8)
    # V weights: fp8 [p, (c n)]; the x64 rides through V and is undone by wo/64
    wvTe = np.ascontiguousarray(
        (64.0 * wv * g_attn[None, :]).T.reshape(NCH, 128, D)
        .transpose(1, 0, 2).reshape(128, NCH * D)).astype(f8)
    woTe = np.ascontiguousarray((wo / 64.0).T.reshape(NPC, 128, D)).astype(bf)
    f8 = ml_dtypes.float8_e4m3
    w1Te = np.ascontiguousarray(
        (64.0 * w1 * g_ffn[None, :]).T.reshape(NCH, 128, NFC, 128)
        .transpose(2, 1, 0, 3).reshape(NFC, 128, D)).astype(f8)
    w3Te = np.ascontiguousarray(
        (64.0 * w3 * g_ffn[None, :]).T.reshape(NCH, 128, NFC, 128)
        .transpose(2, 1, 0, 3).reshape(NFC, 128, D)).astype(f8)
    w2Te = np.ascontiguousarray(
        (64.0 * w2).T.reshape(NFC, 128, NCH, 128).transpose(2, 1, 0, 3)
        .reshape(NCH, 128, F)).astype(f8)
    ones16 = np.ones((128, 128), bf)

    in_maps = []
    for core in range(8):
        b, qt = core // NQT, core % NQT
        # rotate tokens so the local 512-query slice is always quarter 0
        order = (np.arange(S) + qt * T) % S
        xb = x[b][order]                       # [S, D] rotated
        xTe = np.ascontiguousarray(xb.T.reshape(NCH, 128, S)).astype(bf)
        xTe8 = np.ascontiguousarray(xb.T.reshape(NCH, 128, S)).astype(f8)
        # maskT[p, kt*T + q] = mask[b, qt*T + q, k] with k = kt*128 + p in
        # ROTATED key order (keys follow the same rotation as tokens).
        msl = mask[b, qt * T:(qt + 1) * T][:, order]     # [T(q), S(k)] rotated
        maskTe = np.ascontiguousarray(
            msl.T.reshape(NKT, 128, T).transpose(1, 0, 2)
            .reshape(128, NKT * T)).astype(bf)
        in_maps.append({
            "xT": xTe, "xT8": xTe8, "maskT": maskTe,
            "wqT": wqTe, "wkT": wkTe, "wvT": wvTe, "woT": woTe,
            "w1T": w1Te, "w3T": w3Te, "w2T": w2Te,
            "ones16": ones16,
        })
    return in_maps


_NC_CACHE = None


def get_nc():
    global _NC_CACHE
    if _NC_CACHE is None:
        _NC_CACHE = build_nc()
    return _NC_CACHE


def gather_output(results):
    out = np.empty((B, S, D), np.float32)
    for core in range(8):
        b, qt = core // NQT, core % NQT
        o = results[core]["outT"]              # [NCH, 128, T]
        out[b, qt * T:(qt + 1) * T, :] = o.reshape(D, T).T
    return out


def kernel(**inputs):
    from concourse.bass_utils import run_bass_kernel_spmd
    in_maps = prep_inputs(
        np.asarray(inputs["x"]), np.asarray(inputs["mask"]),
        np.asarray(inputs["wq"]), np.asarray(inputs["wk"]),
        np.asarray(inputs["wv"]), np.asarray(inputs["wo"]),
        np.asarray(inputs["w1"]), np.asarray(inputs["w2"]),
        np.asarray(inputs["w3"]),
        np.asarray(inputs["g_attn"]), np.asarray(inputs["g_ffn"]))
    nc = get_nc()
    res = run_bass_kernel_spmd(nc, in_maps, core_ids=list(range(8)))
    return gather_output(res.results)


# revision 39
# speedup vs baseline: 1.0308x; 1.0093x over previous
"""Trainium2 Bass kernel for a pre-norm transformer encoder layer (SwiGLU FFN).

Shapes (hardcoded): x [2, 2048, 768], mask [2, 2048, 2048] int32,
wq/wk/wv/wo [768, 768], w1/w3 [3072, 768], w2 [768, 3072], g_attn/g_ffn [768].

Sharding: 8 cores = 2 batch x 4 query-slices of 512 tokens. Each core
computes K/V for its full batch element (replicated within the group of 4)
and attention + FFN for its own 512 tokens. No collectives.

On-device layout is feature-major ("transposed"): activations [D, tokens].
All matmuls run in bf16 with fp32 PSUM accumulation.

Optimizations over the first working version: bf16 x input; K/Q projected
from unnormalized x with rstd folded in at PSUM evacuation (keeps the PE
stream free of the rmsnorm chain); per-pair softmax normalization (DVE
fast-reciprocal via an SBUF bounce + gpsimd broadcast), emission deferred
into the next pair's g-loop so it never heads the DVE FIFO; head-paired wo
projection (full 128-contract); even/odd score matmuls issued adjacently for
row-tile concurrency; FFN weights prefetched during attention; small
keep-warm matmul blocks bridge the dependency-chain bubbles at phase
transitions so the PE HAM clock gate stays at 2.4 GHz; the K/Q/V
projections and all three FFN matmuls run in fp8e4m3 DoubleRow mode
(2 k-subtiles per matmul; weights scaled x64 into fp8 range and unscaled at
PSUM evacuation / via host-side folds into wo and the final 1/1024 residual
scale); the gpsimd broadcast path is warmed up in stage 1.
"""
import os
import sys

for _p in ("/opt/trn_rl_repo", "/root/.axon_site/_ro/trn_rl_repo"):
    if os.path.isdir(_p) and _p not in sys.path:
        sys.path.append(_p)

import numpy as np
import ml_dtypes

import concourse.bacc as bacc
import concourse.tile as tile
from concourse import mybir

F32 = mybir.dt.float32
BF16 = mybir.dt.bfloat16
F8 = mybir.dt.float8e4
AF = mybir.ActivationFunctionType

B, S, D, H = 2, 2048, 768, 12
DK = D // H            # 64
F = 4 * D              # 3072
T = 512                # local query tokens per core
NCH = D // 128         # 6 feature chunks
NFC = F // 128         # 24 FFN chunks
NKT = S // 128         # 16 key tiles
NQT = S // T           # 4 query slices per batch element
NPC = H // 2           # 6 head pairs
EPS = 1e-5


def build_nc():
    nc = bacc.Bacc("TRN2", target_bir_lowering=False, debug=False, num_devices=8)

    xT = nc.dram_tensor("xT", [NCH, 128, S], BF16, kind="ExternalInput").ap()
    maskT = nc.dram_tensor("maskT", [128, NKT * T], BF16, kind="ExternalInput").ap()
    xT8 = nc.dram_tensor("xT8", [NCH, 128, S], F8, kind="ExternalInput").ap()
    wqT = nc.dram_tensor("wqT", [NCH, 128, D], F8, kind="ExternalInput").ap()
    wkT = nc.dram_tensor("wkT", [NCH, 128, D], F8, kind="ExternalInput").ap()
    wvT = nc.dram_tensor("wvT", [128, NCH * D], F8, kind="ExternalInput").ap()
    woT = nc.dram_tensor("woT", [NPC, 128, D], BF16, kind="ExternalInput").ap()
    w1T = nc.dram_tensor("w1T", [NFC, 128, D], F8, kind="ExternalInput").ap()
    w3T = nc.dram_tensor("w3T", [NFC, 128, D], F8, kind="ExternalInput").ap()
    w2T = nc.dram_tensor("w2T", [NCH, 128, F], F8, kind="ExternalInput").ap()
    ones16 = nc.dram_tensor("ones16", [128, 128], BF16, kind="ExternalInput").ap()

    outT = nc.dram_tensor("outT", [NCH, 128, T], F32, kind="ExternalOutput").ap()
    warm_out = nc.dram_tensor("warm_out", [3, 128, T], BF16,
                              kind="ExternalOutput").ap()

    with tile.TileContext(nc) as tc:
        with tc.tile_pool(name="glob", bufs=1) as Pg:
            ones16_t = Pg.tile([128, 128], BF16, name="ones16_t")
            eps_t = Pg.tile([128, 1], F32, name="eps_t")
            xloc = [Pg.tile([128, T], BF16, name=f"xloc{c}") for c in range(NCH)]
            hT = [Pg.tile([128, T], F32, name=f"hT{c}") for c in range(NCH)]
            attnP = [Pg.tile([128, T], BF16, name=f"attnP{p}") for p in range(NPC)]
            hn8 = Pg.tile([128, NCH * T], F8, name="hn8")
            # first NPRE FFN chunks prefetched during attention; rest streamed
            NPRE = 12
            w1t = [Pg.tile([128, D], F8, name=f"w1_{f}") for f in range(NPRE)]
            w3t = [Pg.tile([128, D], F8, name=f"w3_{f}") for f in range(NPRE)]

            with tc.tile_pool(name="attn", bufs=1) as Pa:
                KT = [Pa.tile([128, S], BF16, name=f"KT{c}") for c in range(NCH)]
                QT = [Pa.tile([128, T], BF16, name=f"QT{c}") for c in range(NCH)]
                VA = [Pa.tile([128, H * (DK + 1)], BF16, name=f"VA{t}")
                      for t in range(NKT)]
                maskT_t = Pa.tile([128, NKT * T], BF16, name="maskT_t")

                # ---------------- stage 1: rmsnorm + Q/K/V projections --------
                with (
                    tc.tile_pool(name="s1", bufs=1) as P1,
                    tc.tile_pool(name="ps1", bufs=1, space="PSUM") as PS1,
                ):
                    wq_t = [P1.tile([128, D], F8, name=f"wq{c}") for c in range(NCH)]
                    wk_t = [P1.tile([128, D], F8, name=f"wk{c}") for c in range(NCH)]
                    wv8 = P1.tile([128, NCH * D], F8, name="wv8")

                    # the ones columns of VA are set once, values filled per tile
                    for t in range(NKT):
                        nc.vector.memset(
                            VA[t][:].rearrange("p (h e) -> p h e",
                                               e=DK + 1)[:, :, DK:DK + 1], 1.0)

                    for qt in range(NQT):
                        sl = slice(qt * T, (qt + 1) * T)
                        xq = [P1.tile([128, T], BF16, name=f"xq{c}", tag=f"xq{c}",
                                      bufs=2) for c in range(NCH)]
                        if qt == 0:
                            nc.sync.dma_start(ones16_t[:], ones16)
                            nc.vector.memset(eps_t[:], EPS)
                        for c in range(NCH):
                            nc.sync.dma_start(xq[c][:], xT[c][:, sl])
                        xq8 = P1.tile([128, NCH * T], F8, tag="xq8", bufs=2,
                                      name=f"xq8_{qt}")
                        for c in range(NCH):
                            nc.sync.dma_start(xq8[:, c * T:(c + 1) * T],
                                              xT8[c][:, sl])
                        if qt == 0:
                            for c in range(NCH):
                                nc.sync.dma_start(wk_t[c][:], wkT[c])
                            for c in range(NCH):
                                nc.sync.dma_start(wq_t[c][:], wqT[c])
                            nc.sync.dma_start(wv8[:], wvT)
                            nc.sync.dma_start(maskT_t[:], maskT)
                        ps_ms = PS1.tile([128, T], F32, tag="ps_ms", name="ps_ms")
                        for c in range(NCH):
                            sq = P1.tile([128, T], BF16, tag="sq", bufs=2,
                                         name=f"sq{qt}_{c}")
                            nc.vector.tensor_mul(sq[:], xq[c][:], xq[c][:])
                            nc.tensor.matmul(ps_ms[:], ones16_t[:], sq[:],
                                             start=(c == 0), stop=(c == NCH - 1))
                        lntmp = P1.tile([128, T], F32, tag="lntmp", bufs=2,
                                        name=f"lntmp{qt}")
                        nc.scalar.activation(lntmp[:], ps_ms[:], AF.Ln,
                                             bias=eps_t[:], scale=1.0 / D)
                        rstd = P1.tile([128, T], F32, tag="rstd", bufs=2,
                                       name=f"rstd{qt}")
                        nc.scalar.activation(rstd[:], lntmp[:], AF.Exp,
                                             scale=-0.5)

                        local = (qt == 0)
                        if local:
                            for c in range(NCH):
                                nc.scalar.copy(xloc[c][:], xq[c][:])
                            # warm up the gpsimd broadcast path: its first
                            # dispatch costs ~5us and would stall attention
                            bcw = P1.tile([DK, T], F32, name="bcw")
                            nc.gpsimd.partition_broadcast(bcw[:],
                                                          lntmp[0:1, :])
                            wbc = P1.tile([1, T], BF16, name="wbc")
                            nc.vector.tensor_copy(wbc[:], bcw[0:1, :])
                            nc.sync.dma_start(warm_out[2][0:1, :], wbc[:])
                        # K/Q projections on UNnormalized x; rstd folded in at
                        # PSUM evacuation (keeps the PE stream free of the
                        # rmsnorm dependency chain).
                        xq8r = xq8[:].rearrange("p (c t) -> p c t", t=T)
                        for do in range(NCH):
                            wkr = wk_t[do][:].rearrange("p (c m) -> p c m", m=128)
                            ps_k = PS1.tile([128, T], F32, tag="ps_k", bufs=3,
                                            name=f"ps_k{qt}_{do}")
                            for j in range(NCH // 2):
                                js = slice(2 * j, 2 * j + 2)
                                nc.tensor.matmul(
                                    ps_k[:], wkr[:, js, :], xq8r[:, js, :],
                                    start=(j == 0), stop=(j == 2),
                                    perf_mode=mybir.MatmulPerfMode.DoubleRow)
                            nc.vector.scalar_tensor_tensor(
                                KT[do][:, sl], ps_k[:], 1.0 / 64, rstd[:],
                                mybir.AluOpType.mult, mybir.AluOpType.mult)
                            if local:
                                wqr = wq_t[do][:].rearrange("p (c m) -> p c m",
                                                            m=128)
                                ps_q = PS1.tile([128, T], F32, tag="ps_v",
                                                bufs=2, name=f"ps_q{do}")
                                for j in range(NCH // 2):
                                    js = slice(2 * j, 2 * j + 2)
                                    nc.tensor.matmul(
                                        ps_q[:], wqr[:, js, :], xq8r[:, js, :],
                                        start=(j == 0), stop=(j == 2),
                                        perf_mode=mybir.MatmulPerfMode.DoubleRow)
                                nc.vector.scalar_tensor_tensor(
                                    QT[do][:], ps_q[:], 1.0 / 64, rstd[:],
                                    mybir.AluOpType.mult, mybir.AluOpType.mult)
                        # normalized tokens (fp8, packed c-major) for V
                        xn8 = P1.tile([128, NCH * T], F8, tag="xn8", bufs=2,
                                      name=f"xn8_{qt}")
                        for c in range(NCH):
                            nc.vector.tensor_mul(xn8[:, c * T:(c + 1) * T],
                                                 xq[c][:], rstd[:])
                        xn8r = xn8[:].rearrange("p (c t) -> p c t", t=T)
                        wv8r = wv8[:].rearrange("p (c n) -> p c n", n=D)
                        # V projection: token-major tiles with ones columns.
                        # V carries the x64 fp8 weight scale; wo is pre-divided
                        for tt in range(4):
                            gt = qt * 4 + tt
                            ps_v = PS1.tile([128, D], F32, tag="ps_v", bufs=2,
                                            name=f"ps_v{gt}")
                            tsl = slice(tt * 128, (tt + 1) * 128)
                            for j in range(NCH // 2):
                                js = slice(2 * j, 2 * j + 2)
                                nc.tensor.matmul(
                                    ps_v[:, 0:512], xn8r[:, js, tsl],
                                    wv8r[:, js, 0:512],
                                    start=(j == 0), stop=(j == 2),
                                    perf_mode=mybir.MatmulPerfMode.DoubleRow)
                                nc.tensor.matmul(
                                    ps_v[:, 512:768], xn8r[:, js, tsl],
                                    wv8r[:, js, 512:768],
                                    start=(j == 0), stop=(j == 2),
                                    perf_mode=mybir.MatmulPerfMode.DoubleRow)
                            nc.vector.tensor_copy(
                                VA[gt][:].rearrange("p (h e) -> p h e",
                                                    e=DK + 1)[:, :, 0:DK],
                                ps_v[:].rearrange("p (h d) -> p h d", d=DK))

                # ---------------- stage 2: attention ------------------------
                # prefetch FFN weights while attention runs
                for f in range(NPRE):
                    nc.sync.dma_start(w1t[f][:], w1T[f])
                    nc.sync.dma_start(w3t[f][:], w3T[f])

                with (
                    tc.tile_pool(name="s2", bufs=1) as P2,
                    tc.tile_pool(name="ps2", bufs=1, space="PSUM") as PS2,
                ):
                    wo_t = [P2.tile([128, D], BF16, name=f"wo{p}")
                            for p in range(NPC)]
                    for p in range(NPC):
                        nc.sync.dma_start(wo_t[p][:], woT[p])

                    def norm_steps(pc, acc):
                        # softmax scale as 4 micro-steps (2 per head) so the
                        # DVE queue never carries the whole chain as one blob
                        # ahead of the next pair's masked-probs multiplies
                        steps = []
                        for h in (2 * pc, 2 * pc + 1):
                            def s1(h=h):
                                srow = P2.tile([1, T], F32, tag="srow", bufs=2,
                                               name=f"srow{h}")
                                nc.vector.tensor_copy(srow[:],
                                                      acc[h][DK:DK + 1, :])
                                rcp = P2.tile([1, T], F32, tag="rcp", bufs=2,
                                              name=f"rcp{h}")
                                nc.vector.reciprocal_approx_fast(rcp[:], srow[:])
                                bc = P2.tile([DK, T], F32, tag="bc", bufs=2,
                                             name=f"bc{h}")
                                nc.gpsimd.partition_broadcast(bc[:], rcp[:])
                                return bc
                            def s2(h=h, pc=pc):
                                r0 = (h % 2) * DK
                                nc.vector.tensor_mul(attnP[pc][r0:r0 + DK, :],
                                                     acc[h][0:DK, :],
                                                     bcs[h][:])
                            steps.append(((h, s1), (None, s2)))
                        # both s1 steps first: each broadcast gets two g-steps
                        # of slack before its multiply enters the DVE queue
                        return [steps[0][0], steps[1][0],
                                steps[0][1], steps[1][1]]

                    bcs = {}
                    pending = []
                    for pc in range(NPC):
                        he, ho = 2 * pc, 2 * pc + 1
                        acc = {h: PS2.tile([128, T], F32, tag="acc", bufs=4,
                                           name=f"acc{h}") for h in (he, ho)}
                        for g in range(8):
                            if 2 <= g <= 5 and pending:
                                h, step = pending.pop(0)
                                r = step()
                                if h is not None:
                                    bcs[h] = r
                            ps = {h: PS2.tile([128, 1024], F32, tag="psc",
                                              bufs=2, name=f"psc{h}_{g}")
                                  for h in (he, ho)}
                            # keep-warm pre-matmuls: fill the ACT-paced PE idle
                            # slots; overwritten by the real score matmuls via
                            # the start=True bank clear, so results are unused
                            npre = 1 if pc < 3 else 0
                            for h in (he, ho):
                                for w in range(npre):
                                    nc.tensor.matmul(
                                        ps[h][:, w * T:(w + 1) * T], ones16_t[:],
                                        xloc[(g + w) % NCH][:],
                                        start=True, stop=True)
                            # adjacent even/odd issue -> row-tile concurrency
                            for j in range(2):
                                kt = 2 * g + j
                                ksl = slice(kt * 128, (kt + 1) * 128)
                                for h in (he, ho):
                                    r0 = (h % 2) * DK
                                    nc.tensor.matmul(
                                        ps[h][:, j * T:(j + 1) * T],
                                        KT[pc][r0:r0 + DK, ksl],
                                        QT[pc][r0:r0 + DK, :],
                                        start=True, stop=True)
                            for h in (he, ho):
                                pr = P2.tile([128, 1024], BF16, tag="probs",
                                             bufs=4, name=f"probs{h}_{g}")
                                nc.scalar.activation(pr[:], ps[h][:], AF.Exp)
                                prm = P2.tile([128, 1024], BF16, tag="probsm",
                                              bufs=4, name=f"probsm{h}_{g}")
                                nc.vector.tensor_mul(
                                    prm[:], pr[:],
                                    maskT_t[:, g * 1024:(g + 1) * 1024])
                                for j in range(2):
                                    kt = 2 * g + j
                                    nc.tensor.matmul(
                                        acc[h][0:DK + 1, :],
                                        VA[kt][:, h * (DK + 1):(h + 1) * (DK + 1)],
                                        prm[:, j * T:(j + 1) * T],
                                        start=(g == 0 and j == 0),
                                        stop=(g == 7 and j == 1))
                        pending = norm_steps(pc, acc)
                    # keep-warm filler: bridges the final normalize chain so
                    # the PE clock gate never sees an idle MID window
                    wps = PS2.tile([128, 1024], F32, tag="psc", bufs=2,
                                   name="warm_ps2")
                    for i in range(16):
                        nc.tensor.matmul(wps[:, 0:T], ones16_t[:], xloc[i % NCH][:],
                                         start=(i == 0), stop=(i == 15))
                    wsb = P2.tile([128, T], BF16, name="warm_sb2")
                    nc.vector.tensor_copy(wsb[:], wps[:, 0:T])
                    nc.sync.dma_start(warm_out[0], wsb[:])
                    for h, step in pending:
                        r = step()
                        if h is not None:
                            bcs[h] = r

                    # wo projection (head pairs, full contract) + residual
                    ps_ms2 = PS2.tile([128, T], F32, tag="acc", bufs=4,
                                      name="ps_ms2")
                    for do in range(NCH):
                        ps_h2 = PS2.tile([128, 1024], F32, tag="psc", bufs=2,
                                         name=f"ps_h2_{do}")
                        for p in range(NPC):
                            nc.tensor.matmul(
                                ps_h2[:, 0:T],
                                wo_t[p][:, do * 128:(do + 1) * 128],
                                attnP[p][:], start=(p == 0), stop=(p == NPC - 1))
                        nc.vector.tensor_add(hT[do][:], ps_h2[:, 0:T], xloc[do][:])
                        sqh = P2.tile([128, T], BF16, tag="sqh", bufs=2,
                                      name=f"sqh{do}")
                        nc.vector.tensor_mul(sqh[:], hT[do][:], hT[do][:])
                        nc.tensor.matmul(ps_ms2[:], ones16_t[:], sqh[:],
                                         start=(do == 0), stop=(do == NCH - 1))
                    lntmp2 = P2.tile([128, T], F32, name="lntmp2")
                    nc.scalar.activation(lntmp2[:], ps_ms2[:], AF.Ln,
                                         bias=eps_t[:], scale=1.0 / D)
                    rstd2 = P2.tile([128, T], F32, name="rstd2")
                    nc.scalar.activation(rstd2[:], lntmp2[:], AF.Exp,
                                         scale=-0.5)
                    for c in range(NCH):
                        nc.vector.tensor_mul(hn8[:, c * T:(c + 1) * T],
                                             hT[c][:], rstd2[:])

            # ------------- stage 3: SwiGLU FFN ------------------------------
            with (
                tc.tile_pool(name="s4", bufs=1) as P4,
                tc.tile_pool(name="ps4", bufs=1, space="PSUM") as PS4,
            ):
                wps4 = PS4.tile([128, T], F32, tag="warm", bufs=1,
                                name="warm_ps4")
                for i in range(12):
                    nc.tensor.matmul(wps4[:], ones16_t[:], xloc[i % NCH][:],
                                     start=(i == 0), stop=(i == 11))
                wsb4 = P4.tile([128, T], BF16, name="warm_sb4")
                nc.vector.tensor_copy(wsb4[:], wps4[:])
                nc.sync.dma_start(warm_out[1], wsb4[:])
                w2t = [P4.tile([128, F], F8, tag="w2_t", bufs=2,
                               name=f"w2_{do}") for do in range(NCH)]
                for do in range(NCH):
                    nc.sync.dma_start(w2t[do][:], w2T[do])
                prod8 = P4.tile([128, NFC * T], F8, name="prod8")
                for f in range(NFC):
                    if f < NPRE:
                        w1f, w3f = w1t[f], w3t[f]
                    else:
                        w1f = P4.tile([128, D], F8, tag="w1s", bufs=6,
                                      name=f"w1s{f}")
                        w3f = P4.tile([128, D], F8, tag="w3s", bufs=6,
                                      name=f"w3s{f}")
                        nc.sync.dma_start(w1f[:], w1T[f])
                        nc.sync.dma_start(w3f[:], w3T[f])
                    ps_u = PS4.tile([128, T], F32, tag="ps_u", bufs=2,
                                    name=f"ps_u{f}")
                    ps_w = PS4.tile([128, T], F32, tag="ps_w", bufs=2,
                                    name=f"ps_w{f}")
                    w1r = w1f[:].rearrange("p (c m) -> p c m", m=128)
                    w3r = w3f[:].rearrange("p (c m) -> p c m", m=128)
                    hnr = hn8[:].rearrange("p (c t) -> p c t", t=T)
                    for j in range(NCH // 2):
                        js = slice(2 * j, 2 * j + 2)
                        nc.tensor.matmul(ps_u[:], w1r[:, js, :], hnr[:, js, :],
                                         start=(j == 0), stop=(j == 2),
                                         perf_mode=mybir.MatmulPerfMode.DoubleRow)
                        nc.tensor.matmul(ps_w[:], w3r[:, js, :], hnr[:, js, :],
                                         start=(j == 0), stop=(j == 2),
                                         perf_mode=mybir.MatmulPerfMode.DoubleRow)
                    # weights carry a x64 fp8 range scale; silu's input scale
                    # undoes it for u, w2 is pre-divided to undo it for w
                    silu = P4.tile([128, T], BF16, tag="silu", bufs=2,
                                   name=f"silu{f}")
                    if os.environ.get("BASS_SIM_SILU") == "1":
                        # CoreSim has no Silu; emulate as u*sigmoid(u)
                        nc.scalar.activation(silu[:], ps_u[:], AF.Sigmoid,
                                             scale=1.0 / 64)
                        nc.vector.tensor_mul(silu[:], silu[:], ps_u[:])
                        nc.vector.scalar_tensor_tensor(
                            prod8[:, f * T:(f + 1) * T], silu[:], 0.25 / 64,
                            ps_w[:], mybir.AluOpType.mult,
                            mybir.AluOpType.mult)
                    else:
                        nc.scalar.activation(silu[:], ps_u[:], AF.Silu,
                                             scale=1.0 / 64)
                        # prod kept at x16 scale so fp8 outliers cannot overflow
                        nc.vector.scalar_tensor_tensor(
                            prod8[:, f * T:(f + 1) * T], silu[:], 0.25,
                            ps_w[:], mybir.AluOpType.mult,
                            mybir.AluOpType.mult)

                # bridge the last-prod dependency chain before the w2 phase
                wps5 = PS4.tile([128, T], F32, tag="warm", bufs=1,
                                name="warm_ps5")
                for i in range(8):
                    nc.tensor.matmul(wps5[:], ones16_t[:], xloc[i % NCH][:],
                                     start=(i == 0), stop=(i == 7))
                wsb5 = P4.tile([128, T], BF16, name="warm_sb5")
                nc.vector.tensor_copy(wsb5[:], wps5[:])
                nc.sync.dma_start(warm_out[0], wsb5[:])
                prod8r = prod8[:].rearrange("p (f t) -> p f t", t=T)
                for do in range(NCH):
                    w2r = w2t[do][:].rearrange("p (f m) -> p f m", m=128)
                    ps_y = PS4.tile([128, T], F32, tag="ps_y", bufs=2,
                                    name=f"ps_y{do}")
                    for j in range(NFC // 2):
                        js = slice(2 * j, 2 * j + 2)
                        nc.tensor.matmul(ps_y[:], w2r[:, js, :],
                                         prod8r[:, js, :],
                                         start=(j == 0), stop=(j == NFC // 2 - 1),
                                         perf_mode=mybir.MatmulPerfMode.DoubleRow)
                    outt = P4.tile([128, T], F32, tag="outt", bufs=2,
                                   name=f"outt{do}")
                    # w2 carries x64, prod x16 -> undo 1/1024
                    nc.vector.scalar_tensor_tensor(
                        outt[:], ps_y[:], 1.0 / 1024, hT[do][:],
                        mybir.AluOpType.mult, mybir.AluOpType.add)
                    nc.sync.dma_start(outT[do], outt[:])

    nc.compile()
    return nc


def prep_inputs(x, mask, wq, wk, wv, wo, w1, w2, w3, g_attn, g_ffn):
    """Build the 8 per-core input maps (host-side sharding + layout)."""
    bf = ml_dtypes.bfloat16
    f8 = ml_dtypes.float8_e4m3
    # K/Q weights: fp8 DoubleRow layout [do, p, (c m)], scaled x64 into fp8
    # range (unscaled at PSUM evacuation); wq also folds 1/sqrt(dk)
    wq_s = 64.0 * wq * (1.0 / np.sqrt(DK))
    wqTe = np.ascontiguousarray(
        (wq_s * g_attn[None, :]).T.reshape(NCH, 128, NCH, 128)
        .transpose(2, 1, 0, 3).reshape(NCH, 128, D)).astype(f8)
    wkTe = np.ascontiguousarray(
        (64.0 * wk * g_attn[None, :]).T.reshape(NCH, 128, NCH, 128)
        .transpose(2, 1, 0, 3).reshape(NCH, 128, D)).astype(f8)
    # V weights: fp8 [p, (c n)]; the x64 rides through V and is undone by wo/64
    wvTe = np.ascontiguousarray(
        (64.0 * wv * g_attn[None, :]).T.reshape(NCH, 128, D)
        .transpose(1, 0, 2).reshape(128, NCH * D)).astype(f8)
    woTe = np.ascontiguousarray((wo / 64.0).T.reshape(NPC, 128, D)).astype(bf)
    f8 = ml_dtypes.float8_e4m3
    w1Te = np.ascontiguousarray(
        (64.0 * w1 * g_ffn[None, :]).T.reshape(NCH, 128, NFC, 128)
        .transpose(2, 1, 0, 3).reshape(NFC, 128, D)).astype(f8)
    w3Te = np.ascontiguousarray(
        (64.0 * w3 * g_ffn[None, :]).T.reshape(NCH, 128, NFC, 128)
        .transpose(2, 1, 0, 3).reshape(NFC, 128, D)).astype(f8)
    w2Te = np.ascontiguousarray(
        (64.0 * w2).T.reshape(NFC, 128, NCH, 128).transpose(2, 1, 0, 3)
        .reshape(NCH, 128, F)).astype(f8)
    ones16 = np.ones((128, 128), bf)

    in_maps = []
    for core in range(8):
        b, qt = core // NQT, core % NQT
        # rotate tokens so the local 512-query slice is always quarter 0
        order = (np.arange(S) + qt * T) % S
        xb = x[b][order]                       # [S, D] rotated
        xTe = np.ascontiguousarray(xb.T.reshape(NCH, 128, S)).astype(bf)
        xTe8 = np.ascontiguousarray(xb.T.reshape(NCH, 128, S)).astype(f8)
        # maskT[p, kt*T + q] = mask[b, qt*T + q, k] with k = kt*128 + p in
        # ROTATED key order (keys follow the same rotation as tokens).
        msl = mask[b, qt * T:(qt + 1) * T][:, order]     # [T(q), S(k)] rotated
        maskTe = np.ascontiguousarray(
            msl.T.reshape(NKT, 128, T).transpose(1, 0, 2)
            .reshape(128, NKT * T)).astype(bf)
        in_maps.append({
            "xT": xTe, "xT8": xTe8, "maskT": maskTe,
            "wqT": wqTe, "wkT": wkTe, "wvT": wvTe, "woT": woTe,
            "w1T": w1Te, "w3T": w3Te, "w2T": w2Te,
            "ones16": ones16,
        })
    return in_maps


_NC_CACHE = None


def get_nc():
    global _NC_CACHE
    if _NC_CACHE is None:
        _NC_CACHE = build_nc()
    return _NC_CACHE


def gather_output(results):
    out = np.empty((B, S, D), np.float32)
    for core in range(8):
        b, qt = core // NQT, core % NQT
        o = results[core]["outT"]              # [NCH, 128, T]
        out[b, qt * T:(qt + 1) * T, :] = o.reshape(D, T).T
    return out


def kernel(**inputs):
    from concourse.bass_utils import run_bass_kernel_spmd
    in_maps = prep_inputs(
        np.asarray(inputs["x"]), np.asarray(inputs["mask"]),
        np.asarray(inputs["wq"]), np.asarray(inputs["wk"]),
        np.asarray(inputs["wv"]), np.asarray(inputs["wo"]),
        np.asarray(inputs["w1"]), np.asarray(inputs["w2"]),
        np.asarray(inputs["w3"]),
        np.asarray(inputs["g_attn"]), np.asarray(inputs["g_ffn"]))
    nc = get_nc()
    res = run_bass_kernel_spmd(nc, in_maps, core_ids=list(range(8)))
    return gather_output(res.results)
